# revision 28
# baseline (speedup 1.0000x reference)
"""
Multi-head masked (causal) attention on 8 Trainium2 NeuronCores.

Sharding: core = 2*b + g  (b = batch 0..3, g = head-group 0..1, 6 heads each).
Each core computes, for its batch b and heads [6g, 6g+6):
    q,k,v projections -> causal attention -> out-projection rows
    [384g, 384g+384) of Wo, output written TRANSPOSED [768, 2048] bf16.
Host gathers: out[b] = (part[2b] + part[2b+1]).T + bo.

Attention is processed per head-PAIR p (3 pairs) and per 512-wide query
tile t (4 tiles).  Scores are computed transposed (S^T[sk, sq]) with the
two heads of a pair occupying PE row-groups 0-1 / 2-3 concurrently
(K=64 each), written into ONE [128, 1024] PSUM tile (half h at columns
[512h, 512h+W)), so a single ACT exp instruction covers both heads.
AV and the softmax-denominator matmuls are M=64:
    A[0:64]  = ctx head0   A[64:128] = ctx head1
    B[0:64]  = denom head0 B[64:128] = denom head1
so normalization is one full-width reciprocal + one full-width multiply
straight out of PSUM (no copies, no partition-shifted PSUM reads).

PSUM budget (8 banks): score pipeline 3 x [128,1024] f32 (6 banks,
shared with projection/out-proj filler units) + A + B (2 banks).

Out-projection: contributions of pairs 0,1 are staged in SBUF (bf16)
as PE filler work; pair-2 contribution is added on top (DVE) and the
single bf16 result is DMA'd out per (t, n-pair) tile.
"""

import numpy as np
import ml_dtypes

import concourse.bass as bass
import concourse.mybir as mybir
import concourse.tile as tile
from concourse import bacc

BF16 = mybir.dt.bfloat16
F32 = mybir.dt.float32

# Problem constants (hardcoded per contract)
B, S, D = 4, 2048, 768
N_HEADS_TOTAL = 12
HD = 64                      # head dim
H = 6                        # local heads per core
NPAIR = H // 2               # head pairs
NC_D = D // 128              # contraction chunks over D (6)
NSK = S // 128               # key blocks (16)
SC = 512                     # query-tile width
NT = S // SC                 # query tiles (4)
SCALE = 1.0 / np.sqrt(HD)


def build_nc():
    nc = bacc.Bacc(None, target_bir_lowering=False)

    xT_d = nc.declare_dram_parameter("xT", [D, S], BF16, isOutput=False)
    wq_d = nc.declare_dram_parameter("wq", [128, NC_D * 384], BF16, isOutput=False)
    wk_d = nc.declare_dram_parameter("wk", [128, NC_D * 384], BF16, isOutput=False)
    wv_d = nc.declare_dram_parameter("wv", [128, NC_D * 384], BF16, isOutput=False)
    wo_d = nc.declare_dram_parameter("wo", [128, 3 * 768], BF16, isOutput=False)
    bqk_d = nc.declare_dram_parameter("bqk", [128, 2 * NPAIR], F32, isOutput=False)
    bv_d = nc.declare_dram_parameter("bv", [128, 384], F32, isOutput=False)
    # [128, 320] = identity(128) | mneg(128) | ones(64)
    const_d = nc.declare_dram_parameter("const", [128, 320], BF16, isOutput=False)
    outT_d = nc.declare_dram_parameter("outT", [D, S], BF16, isOutput=True)
    outT_v = outT_d.rearrange("(k p) c -> p k c", p=128)

    with tile.TileContext(nc) as tc:
        with (
            tc.tile_pool(name="const", bufs=1) as constp,
            tc.tile_pool(name="big", bufs=1) as bigp,
            tc.tile_pool(name="epool", bufs=5) as epool,
            tc.tile_pool(name="rpool", bufs=2) as rpool,
            tc.tile_pool(name="opool", bufs=3) as opool,
            tc.tile_pool(name="spool", bufs=3, space="PSUM") as spool,
            tc.tile_pool(name="cpool", bufs=1, space="PSUM") as cpool,
        ):
            xT_sb = bigp.tile([128, NC_D, S], BF16)
            qT_sb = bigp.tile([128, NPAIR, S], BF16)
            kT_sb = bigp.tile([128, NPAIR, S], BF16)
            v_sb = bigp.tile([128, NSK, H * HD], BF16)
            ctxT_sb = bigp.tile([128, NPAIR, S], BF16)
            stage_sb = bigp.tile([128, D // 128, S], BF16)
            wq_sb = constp.tile([128, NC_D, 384], BF16)
            wk_sb = constp.tile([128, NC_D, 384], BF16)
            wv_sb = constp.tile([128, NC_D, 384], BF16)
            wo_sb = constp.tile([128, 3, 768], BF16)
            bqk_sb = constp.tile([128, 2 * NPAIR], F32)
            bv_sb = constp.tile([128, 384], F32)
            const_sb = constp.tile([128, 320], BF16)
            ident_sb = const_sb[:, 0:128]
            mneg_sb = const_sb[:, 128:256]
            ones_sb = const_sb[:, 256:320]

            # ---- input DMAs.  sync: xT (first 512 cols fine-grained, rest
            # bulk).  scalar: weights/consts, most-urgent first.
            for c in range(NC_D):
                nc.sync.dma_start(xT_sb[:, c, 0:512],
                                  xT_d[c * 128:(c + 1) * 128, 0:512])
            nc.scalar.dma_start(wq_sb[:], wq_d.rearrange("p (c n) -> p c n", n=384))
            nc.scalar.dma_start(wv_sb[:], wv_d.rearrange("p (c n) -> p c n", n=384))
            nc.scalar.dma_start(bqk_sb[:], bqk_d[:])
            nc.scalar.dma_start(const_sb[:], const_d[:])
            for c in range(NC_D):
                nc.sync.dma_start(xT_sb[:, c, 512:1024],
                                  xT_d[c * 128:(c + 1) * 128, 512:1024])
            nc.scalar.dma_start(wk_sb[:], wk_d.rearrange("p (c n) -> p c n", n=384))
            for c in range(NC_D):
                nc.sync.dma_start(xT_sb[:, c, 1024:S],
                                  xT_d[c * 128:(c + 1) * 128, 1024:S])
            nc.scalar.dma_start(bv_sb[:], bv_d[:])
            nc.scalar.dma_start(wo_sb[:], wo_d.rearrange("p (c n) -> p c n", n=768))

            # ---- projection / out-projection units (PE filler work) ----
            def qk_sub(p, which, t):
                """q or k projection for pair p, 512 query cols."""
                w_sb, dst = ((wq_sb, qT_sb), (wk_sb, kT_sb))[which]
                ps = spool.tile([128, 1024], F32, tag="work", name=f"qks{p}{which}{t}")
                for c in range(NC_D):
                    nc.tensor.matmul(
                        ps[:, 0:512],
                        w_sb[:, c, p * 128:(p + 1) * 128],
                        xT_sb[:, c, t * 512:(t + 1) * 512],
                        start=(c == 0), stop=(c == NC_D - 1),
                    )
                nc.vector.tensor_add(
                    out=dst[:, p, t * 512:(t + 1) * 512],
                    in0=ps[:, 0:512],
                    in1=bqk_sb[:, 3 * which + p:3 * which + p + 1]
                        .broadcast_to((128, 512)),
                )

            def qk_unit(p, which, tp):
                """q or k projection for pair p, 1024 query cols (2 chunks)."""
                w_sb, dst = ((wq_sb, qT_sb), (wk_sb, kT_sb))[which]
                ps = spool.tile([128, 1024], F32, tag="work", name=f"qk{p}{which}{tp}")
                base = 1024 * tp
                for i in range(2):
                    for c in range(NC_D):
                        nc.tensor.matmul(
                            ps[:, i * 512:(i + 1) * 512],
                            w_sb[:, c, p * 128:(p + 1) * 128],
                            xT_sb[:, c, base + i * 512:base + i * 512 + 512],
                            start=(c == 0), stop=(c == NC_D - 1),
                        )
                nc.vector.tensor_add(
                    out=dst[:, p, base:base + 1024],
                    in0=ps[:, 0:1024],
                    in1=bqk_sb[:, 3 * which + p:3 * which + p + 1]
                        .broadcast_to((128, 1024)),
                )

            def v_unit(j):
                """v projection for token blocks 2j, 2j+1."""
                ps = spool.tile([128, 1024], F32, tag="work", name=f"v{j}")
                for i in range(2):
                    s = 2 * j + i
                    for c in range(NC_D):
                        nc.tensor.matmul(
                            ps[:, i * 512:i * 512 + 384],
                            xT_sb[:, c, s * 128:(s + 1) * 128],
                            wv_sb[:, c, :],
                            start=(c == 0), stop=(c == NC_D - 1),
                        )
                for i in range(2):
                    nc.vector.tensor_add(
                        out=v_sb[:, 2 * j + i, :],
                        in0=ps[:, i * 512:i * 512 + 384],
                        in1=bv_sb[:],
                    )

            def o01_unit(t, k):
                """stage out-proj contribution of pairs 0,1 for n = 2k, 2k+1."""
                ps = spool.tile([128, 1024], F32, tag="work", name=f"o01_{t}{k}")
                for i in range(2):
                    n = 2 * k + i
                    for c in range(2):
                        nc.tensor.matmul(
                            ps[:, i * 512:(i + 1) * 512],
                            wo_sb[:, c, n * 128:(n + 1) * 128],
                            ctxT_sb[:, c, t * 512:(t + 1) * 512],
                            start=(c == 0), stop=(c == 1),
                        )
                nc.vector.tensor_copy(
                    stage_sb[:, 2 * k:2 * k + 2, t * 512:(t + 1) * 512],
                    ps.rearrange("p (i c) -> p i c", i=2),
                )

            def o2_unit(t, k):
                """add pair-2 contribution onto the stage and DMA out."""
                ps = spool.tile([128, 1024], F32, tag="work", name=f"o2_{t}{k}")
                for i in range(2):
                    n = 2 * k + i
                    nc.tensor.matmul(
                        ps[:, i * 512:(i + 1) * 512],
                        wo_sb[:, 2, n * 128:(n + 1) * 128],
                        ctxT_sb[:, 2, t * 512:(t + 1) * 512],
                        start=True, stop=True,
                    )
                ot = opool.tile([128, 2, 512], BF16, tag="ot", name=f"ot{t}{k}")
                nc.vector.tensor_add(
                    out=ot[:],
                    in0=ps.rearrange("p (i c) -> p i c", i=2),
                    in1=stage_sb[:, 2 * k:2 * k + 2, t * 512:(t + 1) * 512],
                )
                nc.sync.dma_start(
                    outT_v[:, 2 * k:2 * k + 2, t * 512:(t + 1) * 512],
                    ot[:],
                )

            # ---- attention for pair p, query tile t ----
            # split_tail: for the final call, columns [0:256) of the ctx
            # accumulator are final after ik=13 (iks 14,15 only write higher
            # columns), so normalize + pair-2 out-projection + DMA for that
            # column half run inside the call's ACT-paced slack; only the
            # upper half remains as tail work.
            def attention(p, t, fillers, split_tail=False):
                q0 = SC * t
                nik = 4 * (t + 1)
                A = cpool.tile([128, 512], F32, tag="ctxA", name=f"A{p}{t}")
                Bd = cpool.tile([128, 512], F32, tag="ctxB", name=f"B{p}{t}")

                def emit_scores(ik):
                    sq0 = max(q0, 128 * ik)
                    W = q0 + SC - sq0
                    diag = 128 * ik >= q0
                    s_ps = spool.tile([128, 1024], F32, tag="work",
                                      name=f"s{p}{t}{ik}")
                    for half in range(2):
                        hp = slice(64 * half, 64 * half + 64)
                        nc.tensor.matmul(
                            s_ps[:, 512 * half:512 * half + W],
                            kT_sb[hp, p, ik * 128:(ik + 1) * 128],
                            qT_sb[hp, p, sq0:sq0 + W],
                            start=True, stop=not diag,
                            skip_group_check=True,
                        )
                    if diag:
                        # diagonal block: add -30000 to causally-masked
                        # entries (exp -> 0) via one I.T @ mneg matmul per head
                        for half in range(2):
                            nc.tensor.matmul(
                                s_ps[:, 512 * half:512 * half + 128],
                                ident_sb, mneg_sb,
                                start=False, stop=True,
                                skip_group_check=True,
                            )
                    e = epool.tile([128, 1024], BF16, tag="e", name=f"e{p}{t}{ik}")
                    nc.scalar.activation(
                        e.rearrange("p (h c) -> p h c", h=2)[:, :, 0:W],
                        s_ps.rearrange("p (h c) -> p h c", h=2)[:, :, 0:W],
                        mybir.ActivationFunctionType.Exp, scale=float(SCALE),
                    )
                    return e

                e_next = emit_scores(0)
                for ik in range(nik):
                    e = e_next
                    if ik + 1 < nik:
                        e_next = emit_scores(ik + 1)
                    if fillers:
                        npop = -(-len(fillers) // (nik - ik))
                        for _ in range(npop):
                            fillers.pop(0)()
                    sq0 = max(q0, 128 * ik)
                    W = q0 + SC - sq0
                    off = sq0 - q0
                    for half in range(2):
                        h = 2 * p + half
                        nc.tensor.matmul(
                            A[64 * half:64 * half + 64, off:off + W],
                            v_sb[:, ik, 64 * h:64 * h + 64],
                            e[:, 512 * half:512 * half + W],
                            start=(ik == 0), stop=(ik == nik - 1),
                            skip_group_check=True,
                        )
                    for half in range(2):
                        nc.tensor.matmul(
                            Bd[64 * half:64 * half + 64, off:off + W],
                            ones_sb,
                            e[:, 512 * half:512 * half + W],
                            start=(ik == 0), stop=(ik == nik - 1),
                            skip_group_check=True,
                        )

                    if ik == nik - 2:
                        # columns [0:256) are final (the last two key blocks
                        # only write higher columns): normalize them now so
                        # only a 256-wide normalize remains at the boundary
                        if split_tail:
                            norm_and_out2(p, A, Bd, q0, 0)
                        else:
                            norm_half(p, A, Bd, q0, 0)
                if split_tail:
                    norm_and_out2(p, A, Bd, q0, 1)
                else:
                    norm_half(p, A, Bd, q0, 1)

            def norm_half(p, A, Bd, q0, h):
                cs = slice(256 * h, 256 * h + 256)
                rcp = rpool.tile([128, 256], F32, tag="rcpq", name=f"rn{p}{h}")
                nc.vector.reciprocal_approx_fast(rcp[:], Bd[:, cs])
                nc.vector.tensor_mul(ctxT_sb[:, p, q0 + 256 * h:q0 + 256 * h + 256],
                                     A[:, cs], rcp[:])

            def norm_and_out2(p, A, Bd, q0, h):
                cs = slice(256 * h, 256 * h + 256)
                g0 = q0 + 256 * h
                rcp = rpool.tile([128, 256], F32, tag="rcpq", name=f"rq{h}")
                nc.vector.reciprocal_approx_fast(rcp[:], Bd[:, cs])
                nc.vector.tensor_mul(ctxT_sb[:, p, g0:g0 + 256], A[:, cs], rcp[:])
                for k in range(3):
                    ps = spool.tile([128, 1024], F32, tag="work", name=f"oq{h}{k}")
                    for i in range(2):
                        n = 2 * k + i
                        nc.tensor.matmul(
                            ps[:, i * 512:i * 512 + 256],
                            wo_sb[:, 2, n * 128:(n + 1) * 128],
                            ctxT_sb[:, 2, g0:g0 + 256],
                            start=True, stop=True,
                        )
                    ot = opool.tile([128, 2, 256], BF16, tag="otq", name=f"otq{h}{k}")
                    nc.vector.tensor_add(
                        out=ot[:],
                        in0=ps.rearrange("p (i c) -> p i c", i=2)[:, :, 0:256],
                        in1=stage_sb[:, 2 * k:2 * k + 2, g0:g0 + 256],
                    )
                    eng = nc.sync if k % 2 == 0 else nc.scalar
                    eng.dma_start(outT_v[:, 2 * k:2 * k + 2, g0:g0 + 256], ot[:])

            # ---- emission schedule ----
            import functools as ft
            P = ft.partial

            # HAM warmup: dummy matmuls on the first-landed xT chunk fill the
            # otherwise-idle DMA-wait window (~8.5-12us) so the PE clock gate
            # is already released (2.4GHz) when the real prologue starts.
            warm = spool.tile([128, 1024], F32, tag="work", name="warm")
            for r in range(10):
                nc.tensor.matmul(warm[:, (r % 2) * 512:(r % 2) * 512 + 512],
                                 xT_sb[:, 0, 0:128], xT_sb[:, 0, 0:512],
                                 start=True, stop=True, skip_group_check=True)

            # prologue: minimum needed by attention(0,0) / early (0,1)
            qk_sub(0, 0, 0)
            qk_sub(0, 1, 0)
            v_unit(0)
            v_unit(1)

            # early windows: fillers ordered by input-DMA arrival so the
            # attention pipeline-fill gaps are covered by ready work
            # (pair-1 t=0 projections need only the first xT piece + wq/wk)
            FILL = {
                (0, 0): [P(qk_sub, 1, 0, 0), P(qk_sub, 0, 0, 1),
                         P(qk_sub, 0, 1, 1), P(qk_sub, 1, 1, 0)],
                (0, 1): [P(v_unit, 2), P(v_unit, 3), P(qk_unit, 0, 0, 1)],
                (0, 2): [P(qk_unit, 0, 1, 1), P(qk_sub, 1, 0, 1),
                         P(v_unit, 4), P(v_unit, 5)],
                (0, 3): [P(v_unit, 6), P(v_unit, 7), P(qk_sub, 1, 1, 1),
                         P(qk_unit, 1, 0, 1)],
                (1, 0): [P(qk_unit, 1, 1, 1), P(qk_unit, 2, 0, 0)],
                (1, 1): [P(qk_unit, 2, 1, 0), P(qk_unit, 2, 0, 1)],
                (1, 2): [P(qk_unit, 2, 1, 1), P(o01_unit, 0, 0),
                         P(o01_unit, 0, 1), P(o01_unit, 0, 2)],
                (1, 3): [P(o01_unit, 1, 0), P(o01_unit, 1, 1),
                         P(o01_unit, 1, 2), P(o01_unit, 2, 0)],
                (2, 0): [P(o01_unit, 2, 1), P(o01_unit, 2, 2)],
                (2, 1): [P(o2_unit, 0, 0), P(o2_unit, 0, 1), P(o2_unit, 0, 2),
                         P(o01_unit, 3, 0), P(o01_unit, 3, 1)],
                (2, 2): [P(o01_unit, 3, 2), P(o2_unit, 1, 0),
                         P(o2_unit, 1, 1), P(o2_unit, 1, 2)],
                (2, 3): [P(o2_unit, 2, 0), P(o2_unit, 2, 1), P(o2_unit, 2, 2)],
            }
            for p in range(NPAIR):
                for t in range(NT):
                    attention(p, t, FILL[(p, t)],
                              split_tail=(p == 2 and t == 3))
    nc.finalize()
    return nc


_NC_CACHE = None


def _get_nc():
    global _NC_CACHE
    if _NC_CACHE is None:
        _NC_CACHE = build_nc()
    return _NC_CACHE


def make_in_maps(x, Wq, Wk, Wv, bq, bk, bv, Wo, bo):
    bf16 = ml_dtypes.bfloat16
    ident = np.eye(128, dtype=np.float32)
    # mneg[sk, sq] = -30000 where sq < sk (causally masked in the diagonal)
    mneg = np.where(np.arange(128)[None, :] < np.arange(128)[:, None],
                    np.float32(-30000.0), np.float32(0.0))
    ones64 = np.ones((128, 64), np.float32)
    const = np.ascontiguousarray(
        np.concatenate([ident, mneg, ones64], axis=1)).astype(bf16)

    def prep_w(W, hs):
        # [H, 768, 64] -> [768, H*64] -> [128, NC_D, 384] -> [128, NC_D*384]
        w = np.asarray(W[hs]).transpose(1, 0, 2).reshape(D, H * HD)
        w = w.reshape(NC_D, 128, H * HD).transpose(1, 0, 2).reshape(128, -1)
        return np.ascontiguousarray(w).astype(bf16)

    in_maps = []
    for core in range(8):
        b, g = core // 2, core % 2
        hs = slice(6 * g, 6 * g + 6)
        xT = np.ascontiguousarray(np.asarray(x[b]).T).astype(bf16)
        bqk = np.zeros((128, 2 * NPAIR), np.float32)
        for p in range(NPAIR):
            bqk[0:64, p] = bq[6 * g + 2 * p]
            bqk[64:128, p] = bq[6 * g + 2 * p + 1]
            bqk[0:64, NPAIR + p] = bk[6 * g + 2 * p]
            bqk[64:128, NPAIR + p] = bk[6 * g + 2 * p + 1]
        bvr = np.ascontiguousarray(
            np.broadcast_to(np.asarray(bv[hs]).reshape(1, H * HD), (128, H * HD))
        ).astype(np.float32)
        wo = np.asarray(Wo[384 * g:384 * (g + 1), :])
        wo = wo.reshape(3, 128, D).transpose(1, 0, 2).reshape(128, -1)
        wo = np.ascontiguousarray(wo).astype(bf16)
        in_maps.append({
            "xT": xT,
            "wq": prep_w(Wq, hs), "wk": prep_w(Wk, hs), "wv": prep_w(Wv, hs),
            "wo": wo, "bqk": bqk, "bv": bvr, "const": const,
        })
    return in_maps


def gather_out(results, bo):
    out = np.empty((B, S, D), np.float32)
    bo32 = np.asarray(bo, np.float32)
    for b in range(B):
        pT = (results[2 * b]["outT"].astype(np.float32)
              + results[2 * b + 1]["outT"].astype(np.float32))
        out[b] = pT.T + bo32[None, :]
    return out


def kernel(x, Wq, Wk, Wv, bq, bk, bv, Wo, bo):
    from concourse.bass_utils import run_bass_kernel_spmd

    nc = _get_nc()
    in_maps = make_in_maps(x, Wq, Wk, Wv, bq, bk, bv, Wo, bo)
    res = run_bass_kernel_spmd(nc, in_maps, list(range(8)))
    return gather_out(res.results, bo)


# revision 29
# speedup vs baseline: 1.0166x; 1.0166x over previous
"""
Multi-head masked (causal) attention on 8 Trainium2 NeuronCores.

Sharding: core = 2*b + g  (b = batch 0..3, g = head-group 0..1, 6 heads each).
Each core computes, for its batch b and heads [6g, 6g+6):
    q,k,v projections -> causal attention -> out-projection rows
    [384g, 384g+384) of Wo, output written TRANSPOSED [768, 2048] bf16.
Host gathers: out[b] = (part[2b] + part[2b+1]).T + bo.

Attention is processed per head-PAIR p (3 pairs) and per 512-wide query
tile t (4 tiles).  Scores are computed transposed (S^T[sk, sq]) with the
two heads of a pair occupying PE row-groups 0-1 / 2-3 concurrently
(K=64 each), written into ONE [128, 1024] PSUM tile (half h at columns
[512h, 512h+W)), so a single ACT exp instruction covers both heads.
AV and the softmax-denominator matmuls are M=64:
    A[0:64]  = ctx head0   A[64:128] = ctx head1
    B[0:64]  = denom head0 B[64:128] = denom head1
so normalization is one full-width reciprocal + one full-width multiply
straight out of PSUM (no copies, no partition-shifted PSUM reads).

PSUM budget (8 banks): score pipeline 3 x [128,1024] f32 (6 banks,
shared with projection/out-proj filler units) + A + B (2 banks).

Out-projection: contributions of pairs 0,1 are staged in SBUF (bf16)
as PE filler work; pair-2 contribution is added on top (DVE) and the
single bf16 result is DMA'd out per (t, n-pair) tile.
"""

import numpy as np
import ml_dtypes

import concourse.bass as bass
import concourse.mybir as mybir
import concourse.tile as tile
from concourse import bacc

BF16 = mybir.dt.bfloat16
F32 = mybir.dt.float32

# Problem constants (hardcoded per contract)
B, S, D = 4, 2048, 768
N_HEADS_TOTAL = 12
HD = 64                      # head dim
H = 6                        # local heads per core
NPAIR = H // 2               # head pairs
NC_D = D // 128              # contraction chunks over D (6)
NSK = S // 128               # key blocks (16)
SC = 512                     # query-tile width
NT = S // SC                 # query tiles (4)
SCALE = 1.0 / np.sqrt(HD)


def build_nc():
    nc = bacc.Bacc(None, target_bir_lowering=False)

    xT_d = nc.declare_dram_parameter("xT", [D, S], BF16, isOutput=False)
    wq_d = nc.declare_dram_parameter("wq", [128, NC_D * 384], BF16, isOutput=False)
    wk_d = nc.declare_dram_parameter("wk", [128, NC_D * 384], BF16, isOutput=False)
    wv_d = nc.declare_dram_parameter("wv", [128, NC_D * 384], BF16, isOutput=False)
    wo_d = nc.declare_dram_parameter("wo", [128, 3 * 768], BF16, isOutput=False)
    bqk_d = nc.declare_dram_parameter("bqk", [128, 2 * NPAIR], F32, isOutput=False)
    bv_d = nc.declare_dram_parameter("bv", [128, 384], F32, isOutput=False)
    # [128, 320] = identity(128) | mneg(128) | ones(64)
    const_d = nc.declare_dram_parameter("const", [128, 320], BF16, isOutput=False)
    outT_d = nc.declare_dram_parameter("outT", [D, S], BF16, isOutput=True)
    outT_v = outT_d.rearrange("(k p) c -> p k c", p=128)

    with tile.TileContext(nc) as tc:
        with (
            tc.tile_pool(name="const", bufs=1) as constp,
            tc.tile_pool(name="big", bufs=1) as bigp,
            tc.tile_pool(name="epool", bufs=5) as epool,
            tc.tile_pool(name="rpool", bufs=2) as rpool,
            tc.tile_pool(name="opool", bufs=3) as opool,
            tc.tile_pool(name="spool", bufs=3, space="PSUM") as spool,
            tc.tile_pool(name="cpool", bufs=1, space="PSUM") as cpool,
        ):
            xT_sb = bigp.tile([128, NC_D, S], BF16)
            qT_sb = bigp.tile([128, NPAIR, S], BF16)
            kT_sb = bigp.tile([128, NPAIR, S], BF16)
            v_sb = bigp.tile([128, NSK, H * HD], BF16)
            ctxT_sb = bigp.tile([128, NPAIR, S], BF16)
            stage_sb = bigp.tile([128, D // 128, S], BF16)
            wq_sb = constp.tile([128, NC_D, 384], BF16)
            wk_sb = constp.tile([128, NC_D, 384], BF16)
            wv_sb = constp.tile([128, NC_D, 384], BF16)
            wo_sb = constp.tile([128, 3, 768], BF16)
            bqk_sb = constp.tile([128, 2 * NPAIR], F32)
            bv_sb = constp.tile([128, 384], F32)
            const_sb = constp.tile([128, 320], BF16)
            ident_sb = const_sb[:, 0:128]
            mneg_sb = const_sb[:, 128:256]
            ones_sb = const_sb[:, 256:320]

            # ---- input DMAs.  sync: xT (first 512 cols fine-grained, rest
            # bulk).  scalar: weights/consts, most-urgent first.
            for c in range(NC_D):
                nc.sync.dma_start(xT_sb[:, c, 0:512],
                                  xT_d[c * 128:(c + 1) * 128, 0:512])
            nc.scalar.dma_start(wq_sb[:], wq_d.rearrange("p (c n) -> p c n", n=384))
            nc.scalar.dma_start(wv_sb[:], wv_d.rearrange("p (c n) -> p c n", n=384))
            nc.scalar.dma_start(bqk_sb[:], bqk_d[:])
            nc.scalar.dma_start(const_sb[:], const_d[:])
            for c in range(NC_D):
                nc.sync.dma_start(xT_sb[:, c, 512:1024],
                                  xT_d[c * 128:(c + 1) * 128, 512:1024])
            nc.scalar.dma_start(wk_sb[:], wk_d.rearrange("p (c n) -> p c n", n=384))
            for c in range(NC_D):
                nc.sync.dma_start(xT_sb[:, c, 1024:S],
                                  xT_d[c * 128:(c + 1) * 128, 1024:S])
            nc.scalar.dma_start(bv_sb[:], bv_d[:])
            nc.scalar.dma_start(wo_sb[:], wo_d.rearrange("p (c n) -> p c n", n=768))

            # ---- projection / out-projection units (PE filler work) ----
            def qk_sub(p, which, t):
                """q or k projection for pair p, 512 query cols."""
                w_sb, dst = ((wq_sb, qT_sb), (wk_sb, kT_sb))[which]
                ps = spool.tile([128, 1024], F32, tag="work", name=f"qks{p}{which}{t}")
                for c in range(NC_D):
                    nc.tensor.matmul(
                        ps[:, 0:512],
                        w_sb[:, c, p * 128:(p + 1) * 128],
                        xT_sb[:, c, t * 512:(t + 1) * 512],
                        start=(c == 0), stop=(c == NC_D - 1),
                    )
                nc.vector.tensor_add(
                    out=dst[:, p, t * 512:(t + 1) * 512],
                    in0=ps[:, 0:512],
                    in1=bqk_sb[:, 3 * which + p:3 * which + p + 1]
                        .broadcast_to((128, 512)),
                )

            def qk_unit(p, which, tp):
                """q or k projection for pair p, 1024 query cols (2 chunks)."""
                w_sb, dst = ((wq_sb, qT_sb), (wk_sb, kT_sb))[which]
                ps = spool.tile([128, 1024], F32, tag="work", name=f"qk{p}{which}{tp}")
                base = 1024 * tp
                for i in range(2):
                    for c in range(NC_D):
                        nc.tensor.matmul(
                            ps[:, i * 512:(i + 1) * 512],
                            w_sb[:, c, p * 128:(p + 1) * 128],
                            xT_sb[:, c, base + i * 512:base + i * 512 + 512],
                            start=(c == 0), stop=(c == NC_D - 1),
                        )
                nc.vector.tensor_add(
                    out=dst[:, p, base:base + 1024],
                    in0=ps[:, 0:1024],
                    in1=bqk_sb[:, 3 * which + p:3 * which + p + 1]
                        .broadcast_to((128, 1024)),
                )

            def v_unit(j):
                """v projection for token blocks 2j, 2j+1."""
                ps = spool.tile([128, 1024], F32, tag="work", name=f"v{j}")
                for i in range(2):
                    s = 2 * j + i
                    for c in range(NC_D):
                        nc.tensor.matmul(
                            ps[:, i * 512:i * 512 + 384],
                            xT_sb[:, c, s * 128:(s + 1) * 128],
                            wv_sb[:, c, :],
                            start=(c == 0), stop=(c == NC_D - 1),
                        )
                for i in range(2):
                    nc.vector.tensor_add(
                        out=v_sb[:, 2 * j + i, :],
                        in0=ps[:, i * 512:i * 512 + 384],
                        in1=bv_sb[:],
                    )

            def o01_unit(t, k):
                """stage out-proj contribution of pairs 0,1 for n = 2k, 2k+1."""
                ps = spool.tile([128, 1024], F32, tag="work", name=f"o01_{t}{k}")
                for i in range(2):
                    n = 2 * k + i
                    for c in range(2):
                        nc.tensor.matmul(
                            ps[:, i * 512:(i + 1) * 512],
                            wo_sb[:, c, n * 128:(n + 1) * 128],
                            ctxT_sb[:, c, t * 512:(t + 1) * 512],
                            start=(c == 0), stop=(c == 1),
                        )
                nc.vector.tensor_copy(
                    stage_sb[:, 2 * k:2 * k + 2, t * 512:(t + 1) * 512],
                    ps.rearrange("p (i c) -> p i c", i=2),
                )

            def o2_unit(t, k):
                """add pair-2 contribution onto the stage and DMA out."""
                ps = spool.tile([128, 1024], F32, tag="work", name=f"o2_{t}{k}")
                for i in range(2):
                    n = 2 * k + i
                    nc.tensor.matmul(
                        ps[:, i * 512:(i + 1) * 512],
                        wo_sb[:, 2, n * 128:(n + 1) * 128],
                        ctxT_sb[:, 2, t * 512:(t + 1) * 512],
                        start=True, stop=True,
                    )
                ot = opool.tile([128, 2, 512], BF16, tag="ot", name=f"ot{t}{k}")
                nc.vector.tensor_add(
                    out=ot[:],
                    in0=ps.rearrange("p (i c) -> p i c", i=2),
                    in1=stage_sb[:, 2 * k:2 * k + 2, t * 512:(t + 1) * 512],
                )
                nc.sync.dma_start(
                    outT_v[:, 2 * k:2 * k + 2, t * 512:(t + 1) * 512],
                    ot[:],
                )

            # ---- attention for pair p, query tile t ----
            # split_tail: for the final call, columns [0:256) of the ctx
            # accumulator are final after ik=13 (iks 14,15 only write higher
            # columns), so normalize + pair-2 out-projection + DMA for that
            # column half run inside the call's ACT-paced slack; only the
            # upper half remains as tail work.
            def attention(p, t, fillers, split_tail=False):
                q0 = SC * t
                nik = 4 * (t + 1)
                A = cpool.tile([128, 512], F32, tag="ctxA", name=f"A{p}{t}")
                Bd = cpool.tile([128, 512], F32, tag="ctxB", name=f"B{p}{t}")

                def emit_scores(ik):
                    sq0 = max(q0, 128 * ik)
                    W = q0 + SC - sq0
                    diag = 128 * ik >= q0
                    s_ps = spool.tile([128, 1024], F32, tag="work",
                                      name=f"s{p}{t}{ik}")
                    for half in range(2):
                        hp = slice(64 * half, 64 * half + 64)
                        nc.tensor.matmul(
                            s_ps[:, 512 * half:512 * half + W],
                            kT_sb[hp, p, ik * 128:(ik + 1) * 128],
                            qT_sb[hp, p, sq0:sq0 + W],
                            start=True, stop=not diag,
                            skip_group_check=True,
                        )
                    if diag:
                        # diagonal block: add -30000 to causally-masked
                        # entries (exp -> 0) via one I.T @ mneg matmul per head
                        for half in range(2):
                            nc.tensor.matmul(
                                s_ps[:, 512 * half:512 * half + 128],
                                ident_sb, mneg_sb,
                                start=False, stop=True,
                                skip_group_check=True,
                            )
                    e = epool.tile([128, 1024], BF16, tag="e", name=f"e{p}{t}{ik}")
                    nc.scalar.activation(
                        e.rearrange("p (h c) -> p h c", h=2)[:, :, 0:W],
                        s_ps.rearrange("p (h c) -> p h c", h=2)[:, :, 0:W],
                        mybir.ActivationFunctionType.Exp, scale=float(SCALE),
                    )
                    return e

                e_next = emit_scores(0)
                for ik in range(nik):
                    e = e_next
                    if ik + 1 < nik:
                        e_next = emit_scores(ik + 1)
                    if fillers:
                        npop = -(-len(fillers) // (nik - ik))
                        for _ in range(npop):
                            fillers.pop(0)()
                    sq0 = max(q0, 128 * ik)
                    W = q0 + SC - sq0
                    off = sq0 - q0
                    for half in range(2):
                        h = 2 * p + half
                        nc.tensor.matmul(
                            A[64 * half:64 * half + 64, off:off + W],
                            v_sb[:, ik, 64 * h:64 * h + 64],
                            e[:, 512 * half:512 * half + W],
                            start=(ik == 0), stop=(ik == nik - 1),
                            skip_group_check=True,
                        )
                    for half in range(2):
                        nc.tensor.matmul(
                            Bd[64 * half:64 * half + 64, off:off + W],
                            ones_sb,
                            e[:, 512 * half:512 * half + W],
                            start=(ik == 0), stop=(ik == nik - 1),
                            skip_group_check=True,
                        )

                    if ik == nik - 2:
                        # columns [0:256) are final (the last two key blocks
                        # only write higher columns): normalize them now so
                        # only a 256-wide normalize remains at the boundary
                        if split_tail:
                            norm_and_out2(p, A, Bd, q0, 0)
                        else:
                            norm_half(p, A, Bd, q0, 0)
                if split_tail:
                    norm_and_out2(p, A, Bd, q0, 1)
                else:
                    norm_half(p, A, Bd, q0, 1)

            def norm_half(p, A, Bd, q0, h):
                cs = slice(256 * h, 256 * h + 256)
                rcp = rpool.tile([128, 256], F32, tag="rcpq", name=f"rn{p}{h}")
                nc.vector.reciprocal_approx_fast(rcp[:], Bd[:, cs])
                nc.vector.tensor_mul(ctxT_sb[:, p, q0 + 256 * h:q0 + 256 * h + 256],
                                     A[:, cs], rcp[:])

            def norm_and_out2(p, A, Bd, q0, h):
                cs = slice(256 * h, 256 * h + 256)
                g0 = q0 + 256 * h
                rcp = rpool.tile([128, 256], F32, tag="rcpq", name=f"rq{h}")
                nc.vector.reciprocal_approx_fast(rcp[:], Bd[:, cs])
                nc.vector.tensor_mul(ctxT_sb[:, p, g0:g0 + 256], A[:, cs], rcp[:])
                for k in range(3):
                    ps = spool.tile([128, 1024], F32, tag="work", name=f"oq{h}{k}")
                    for i in range(2):
                        n = 2 * k + i
                        nc.tensor.matmul(
                            ps[:, i * 512:i * 512 + 256],
                            wo_sb[:, 2, n * 128:(n + 1) * 128],
                            ctxT_sb[:, 2, g0:g0 + 256],
                            start=True, stop=True,
                        )
                    ot = opool.tile([128, 2, 256], BF16, tag="otq", name=f"otq{h}{k}")
                    nc.vector.tensor_add(
                        out=ot[:],
                        in0=ps.rearrange("p (i c) -> p i c", i=2)[:, :, 0:256],
                        in1=stage_sb[:, 2 * k:2 * k + 2, g0:g0 + 256],
                    )
                    eng = nc.sync if k % 2 == 0 else nc.scalar
                    eng.dma_start(outT_v[:, 2 * k:2 * k + 2, g0:g0 + 256], ot[:])

            # ---- emission schedule ----
            import functools as ft
            P = ft.partial

            # HAM warmup: dummy matmuls on the first-landed xT chunk fill the
            # otherwise-idle DMA-wait window (~8.5-12us) so the PE clock gate
            # is already released (2.4GHz) when the real prologue starts.
            warm = spool.tile([128, 1024], F32, tag="work", name="warm")
            for _ in range(10):
                nc.tensor.matmul(warm[:, 0:512],
                                 xT_sb[:, 0, 0:128], xT_sb[:, 0, 0:512],
                                 start=True, stop=True)

            # prologue: minimum needed by attention(0,0) / early (0,1)
            qk_sub(0, 0, 0)
            qk_sub(0, 1, 0)
            v_unit(0)
            v_unit(1)

            # early windows: fillers ordered by input-DMA arrival so the
            # attention pipeline-fill gaps are covered by ready work
            # (pair-1 t=0 projections need only the first xT piece + wq/wk)
            FILL = {
                (0, 0): [P(qk_sub, 1, 0, 0), P(qk_sub, 0, 0, 1),
                         P(qk_sub, 0, 1, 1), P(qk_sub, 1, 1, 0)],
                (0, 1): [P(v_unit, 2), P(v_unit, 3), P(qk_unit, 0, 0, 1)],
                (0, 2): [P(qk_unit, 0, 1, 1), P(qk_sub, 1, 0, 1),
                         P(v_unit, 4), P(v_unit, 5)],
                (0, 3): [P(v_unit, 6), P(v_unit, 7), P(qk_sub, 1, 1, 1),
                         P(qk_unit, 1, 0, 1)],
                (1, 0): [P(qk_unit, 1, 1, 1), P(qk_unit, 2, 0, 0)],
                (1, 1): [P(qk_unit, 2, 1, 0), P(qk_unit, 2, 0, 1)],
                (1, 2): [P(qk_unit, 2, 1, 1), P(o01_unit, 0, 0),
                         P(o01_unit, 0, 1), P(o01_unit, 0, 2)],
                (1, 3): [P(o01_unit, 1, 0), P(o01_unit, 1, 1),
                         P(o01_unit, 1, 2), P(o01_unit, 2, 0)],
                (2, 0): [P(o01_unit, 2, 1), P(o01_unit, 2, 2)],
                (2, 1): [P(o2_unit, 0, 0), P(o2_unit, 0, 1), P(o2_unit, 0, 2),
                         P(o01_unit, 3, 0), P(o01_unit, 3, 1)],
                (2, 2): [P(o01_unit, 3, 2), P(o2_unit, 1, 0),
                         P(o2_unit, 1, 1), P(o2_unit, 1, 2)],
                (2, 3): [P(o2_unit, 2, 0), P(o2_unit, 2, 1), P(o2_unit, 2, 2)],
            }
            for p in range(NPAIR):
                for t in range(NT):
                    attention(p, t, FILL[(p, t)],
                              split_tail=(p == 2 and t == 3))
    nc.finalize()
    return nc


_NC_CACHE = None


def _get_nc():
    global _NC_CACHE
    if _NC_CACHE is None:
        _NC_CACHE = build_nc()
    return _NC_CACHE


def make_in_maps(x, Wq, Wk, Wv, bq, bk, bv, Wo, bo):
    bf16 = ml_dtypes.bfloat16
    ident = np.eye(128, dtype=np.float32)
    # mneg[sk, sq] = -30000 where sq < sk (causally masked in the diagonal)
    mneg = np.where(np.arange(128)[None, :] < np.arange(128)[:, None],
                    np.float32(-30000.0), np.float32(0.0))
    ones64 = np.ones((128, 64), np.float32)
    const = np.ascontiguousarray(
        np.concatenate([ident, mneg, ones64], axis=1)).astype(bf16)

    def prep_w(W, hs):
        # [H, 768, 64] -> [768, H*64] -> [128, NC_D, 384] -> [128, NC_D*384]
        w = np.asarray(W[hs]).transpose(1, 0, 2).reshape(D, H * HD)
        w = w.reshape(NC_D, 128, H * HD).transpose(1, 0, 2).reshape(128, -1)
        return np.ascontiguousarray(w).astype(bf16)

    in_maps = []
    for core in range(8):
        b, g = core // 2, core % 2
        hs = slice(6 * g, 6 * g + 6)
        xT = np.ascontiguousarray(np.asarray(x[b]).T).astype(bf16)
        bqk = np.zeros((128, 2 * NPAIR), np.float32)
        for p in range(NPAIR):
            bqk[0:64, p] = bq[6 * g + 2 * p]
            bqk[64:128, p] = bq[6 * g + 2 * p + 1]
            bqk[0:64, NPAIR + p] = bk[6 * g + 2 * p]
            bqk[64:128, NPAIR + p] = bk[6 * g + 2 * p + 1]
        bvr = np.ascontiguousarray(
            np.broadcast_to(np.asarray(bv[hs]).reshape(1, H * HD), (128, H * HD))
        ).astype(np.float32)
        wo = np.asarray(Wo[384 * g:384 * (g + 1), :])
        wo = wo.reshape(3, 128, D).transpose(1, 0, 2).reshape(128, -1)
        wo = np.ascontiguousarray(wo).astype(bf16)
        in_maps.append({
            "xT": xT,
            "wq": prep_w(Wq, hs), "wk": prep_w(Wk, hs), "wv": prep_w(Wv, hs),
            "wo": wo, "bqk": bqk, "bv": bvr, "const": const,
        })
    return in_maps


def gather_out(results, bo):
    out = np.empty((B, S, D), np.float32)
    bo32 = np.asarray(bo, np.float32)
    for b in range(B):
        pT = (results[2 * b]["outT"].astype(np.float32)
              + results[2 * b + 1]["outT"].astype(np.float32))
        out[b] = pT.T + bo32[None, :]
    return out


def kernel(x, Wq, Wk, Wv, bq, bk, bv, Wo, bo):
    from concourse.bass_utils import run_bass_kernel_spmd

    nc = _get_nc()
    in_maps = make_in_maps(x, Wq, Wk, Wv, bq, bk, bv, Wo, bo)
    res = run_bass_kernel_spmd(nc, in_maps, list(range(8)))
    return gather_out(res.results, bo)


# revision 30
# speedup vs baseline: 1.0191x; 1.0024x over previous
"""
Multi-head masked (causal) attention on 8 Trainium2 NeuronCores.

Sharding: core = 2*b + g  (b = batch 0..3, g = head-group 0..1, 6 heads each).
Each core computes, for its batch b and heads [6g, 6g+6):
    q,k,v projections -> causal attention -> out-projection rows
    [384g, 384g+384) of Wo, output written TRANSPOSED [768, 2048] bf16.
Host gathers: out[b] = (part[2b] + part[2b+1]).T + bo.

Attention is processed per head-PAIR p (3 pairs) and per 512-wide query
tile t (4 tiles).  Scores are computed transposed (S^T[sk, sq]) with the
two heads of a pair occupying PE row-groups 0-1 / 2-3 concurrently
(K=64 each), written into ONE [128, 1024] PSUM tile (half h at columns
[512h, 512h+W)), so a single ACT exp instruction covers both heads.
AV and the softmax-denominator matmuls are M=64:
    A[0:64]  = ctx head0   A[64:128] = ctx head1
    B[0:64]  = denom head0 B[64:128] = denom head1
so normalization is one full-width reciprocal + one full-width multiply
straight out of PSUM (no copies, no partition-shifted PSUM reads).

PSUM budget (8 banks): score pipeline 3 x [128,1024] f32 (6 banks,
shared with projection/out-proj filler units) + A + B (2 banks).

Out-projection: contributions of pairs 0,1 are staged in SBUF (bf16)
as PE filler work; pair-2 contribution is added on top (DVE) and the
single bf16 result is DMA'd out per (t, n-pair) tile.
"""

import numpy as np
import ml_dtypes

import concourse.bass as bass
import concourse.mybir as mybir
import concourse.tile as tile
from concourse import bacc

BF16 = mybir.dt.bfloat16
F32 = mybir.dt.float32

# Problem constants (hardcoded per contract)
B, S, D = 4, 2048, 768
N_HEADS_TOTAL = 12
HD = 64                      # head dim
H = 6                        # local heads per core
NPAIR = H // 2               # head pairs
NC_D = D // 128              # contraction chunks over D (6)
NSK = S // 128               # key blocks (16)
SC = 512                     # query-tile width
NT = S // SC                 # query tiles (4)
SCALE = 1.0 / np.sqrt(HD)


def build_nc():
    nc = bacc.Bacc(None, target_bir_lowering=False)

    xT_d = nc.declare_dram_parameter("xT", [D, S], BF16, isOutput=False)
    wq_d = nc.declare_dram_parameter("wq", [128, NC_D * 384], BF16, isOutput=False)
    wk_d = nc.declare_dram_parameter("wk", [128, NC_D * 384], BF16, isOutput=False)
    wv_d = nc.declare_dram_parameter("wv", [128, NC_D * 384], BF16, isOutput=False)
    wo_d = nc.declare_dram_parameter("wo", [128, 3 * 768], BF16, isOutput=False)
    bqk_d = nc.declare_dram_parameter("bqk", [128, 2 * NPAIR], F32, isOutput=False)
    bv_d = nc.declare_dram_parameter("bv", [128, 384], F32, isOutput=False)
    # [128, 320] = identity(128) | mneg(128) | ones(64)
    const_d = nc.declare_dram_parameter("const", [128, 320], BF16, isOutput=False)
    outT_d = nc.declare_dram_parameter("outT", [D, S], BF16, isOutput=True)
    outT_v = outT_d.rearrange("(k p) c -> p k c", p=128)

    with tile.TileContext(nc) as tc:
        with (
            tc.tile_pool(name="const", bufs=1) as constp,
            tc.tile_pool(name="big", bufs=1) as bigp,
            tc.tile_pool(name="epool", bufs=5) as epool,
            tc.tile_pool(name="rpool", bufs=2) as rpool,
            tc.tile_pool(name="opool", bufs=3) as opool,
            tc.tile_pool(name="spool", bufs=3, space="PSUM") as spool,
            tc.tile_pool(name="cpool", bufs=1, space="PSUM") as cpool,
        ):
            xT_sb = bigp.tile([128, NC_D, S], BF16)
            qT_sb = bigp.tile([128, NPAIR, S], BF16)
            kT_sb = bigp.tile([128, NPAIR, S], BF16)
            v_sb = bigp.tile([128, NSK, H * HD], BF16)
            ctxT_sb = bigp.tile([128, NPAIR, S], BF16)
            stage_sb = bigp.tile([128, D // 128, S], BF16)
            wq_sb = constp.tile([128, NC_D, 384], BF16)
            wk_sb = constp.tile([128, NC_D, 384], BF16)
            wv_sb = constp.tile([128, NC_D, 384], BF16)
            wo_sb = constp.tile([128, 3, 768], BF16)
            bqk_sb = constp.tile([128, 2 * NPAIR], F32)
            bv_sb = constp.tile([128, 384], F32)
            const_sb = constp.tile([128, 320], BF16)
            ident_sb = const_sb[:, 0:128]
            mneg_sb = const_sb[:, 128:256]
            ones_sb = const_sb[:, 256:320]

            # ---- input DMAs.  sync: xT (first 512 cols fine-grained, rest
            # bulk).  scalar: weights/consts, most-urgent first.
            for c in range(NC_D):
                nc.sync.dma_start(xT_sb[:, c, 0:512],
                                  xT_d[c * 128:(c + 1) * 128, 0:512])
            nc.scalar.dma_start(wq_sb[:], wq_d.rearrange("p (c n) -> p c n", n=384))
            nc.scalar.dma_start(wv_sb[:], wv_d.rearrange("p (c n) -> p c n", n=384))
            nc.scalar.dma_start(bqk_sb[:], bqk_d[:])
            nc.scalar.dma_start(const_sb[:], const_d[:])
            for c in range(NC_D):
                nc.sync.dma_start(xT_sb[:, c, 512:1024],
                                  xT_d[c * 128:(c + 1) * 128, 512:1024])
            nc.scalar.dma_start(wk_sb[:], wk_d.rearrange("p (c n) -> p c n", n=384))
            for c in range(NC_D):
                nc.sync.dma_start(xT_sb[:, c, 1024:S],
                                  xT_d[c * 128:(c + 1) * 128, 1024:S])
            nc.scalar.dma_start(bv_sb[:], bv_d[:])
            nc.scalar.dma_start(wo_sb[:], wo_d.rearrange("p (c n) -> p c n", n=768))

            # ---- projection / out-projection units (PE filler work) ----
            def qk_sub(p, which, t):
                """q or k projection for pair p, 512 query cols."""
                w_sb, dst = ((wq_sb, qT_sb), (wk_sb, kT_sb))[which]
                ps = spool.tile([128, 1024], F32, tag="work", name=f"qks{p}{which}{t}")
                for c in range(NC_D):
                    nc.tensor.matmul(
                        ps[:, 0:512],
                        w_sb[:, c, p * 128:(p + 1) * 128],
                        xT_sb[:, c, t * 512:(t + 1) * 512],
                        start=(c == 0), stop=(c == NC_D - 1),
                    )
                nc.vector.tensor_add(
                    out=dst[:, p, t * 512:(t + 1) * 512],
                    in0=ps[:, 0:512],
                    in1=bqk_sb[:, 3 * which + p:3 * which + p + 1]
                        .broadcast_to((128, 512)),
                )

            def qk_unit(p, which, tp):
                """q or k projection for pair p, 1024 query cols (2 chunks)."""
                w_sb, dst = ((wq_sb, qT_sb), (wk_sb, kT_sb))[which]
                ps = spool.tile([128, 1024], F32, tag="work", name=f"qk{p}{which}{tp}")
                base = 1024 * tp
                for i in range(2):
                    for c in range(NC_D):
                        nc.tensor.matmul(
                            ps[:, i * 512:(i + 1) * 512],
                            w_sb[:, c, p * 128:(p + 1) * 128],
                            xT_sb[:, c, base + i * 512:base + i * 512 + 512],
                            start=(c == 0), stop=(c == NC_D - 1),
                        )
                nc.vector.tensor_add(
                    out=dst[:, p, base:base + 1024],
                    in0=ps[:, 0:1024],
                    in1=bqk_sb[:, 3 * which + p:3 * which + p + 1]
                        .broadcast_to((128, 1024)),
                )

            def v_unit(j):
                """v projection for token blocks 2j, 2j+1."""
                ps = spool.tile([128, 1024], F32, tag="work", name=f"v{j}")
                for i in range(2):
                    s = 2 * j + i
                    for c in range(NC_D):
                        nc.tensor.matmul(
                            ps[:, i * 512:i * 512 + 384],
                            xT_sb[:, c, s * 128:(s + 1) * 128],
                            wv_sb[:, c, :],
                            start=(c == 0), stop=(c == NC_D - 1),
                        )
                for i in range(2):
                    nc.vector.tensor_add(
                        out=v_sb[:, 2 * j + i, :],
                        in0=ps[:, i * 512:i * 512 + 384],
                        in1=bv_sb[:],
                    )

            def o01_unit(t, k):
                """stage out-proj contribution of pairs 0,1 for n = 2k, 2k+1."""
                ps = spool.tile([128, 1024], F32, tag="work", name=f"o01_{t}{k}")
                for i in range(2):
                    n = 2 * k + i
                    for c in range(2):
                        nc.tensor.matmul(
                            ps[:, i * 512:(i + 1) * 512],
                            wo_sb[:, c, n * 128:(n + 1) * 128],
                            ctxT_sb[:, c, t * 512:(t + 1) * 512],
                            start=(c == 0), stop=(c == 1),
                        )
                nc.vector.tensor_copy(
                    stage_sb[:, 2 * k:2 * k + 2, t * 512:(t + 1) * 512],
                    ps.rearrange("p (i c) -> p i c", i=2),
                )

            def o2_unit(t, k):
                """add pair-2 contribution onto the stage and DMA out."""
                ps = spool.tile([128, 1024], F32, tag="work", name=f"o2_{t}{k}")
                for i in range(2):
                    n = 2 * k + i
                    nc.tensor.matmul(
                        ps[:, i * 512:(i + 1) * 512],
                        wo_sb[:, 2, n * 128:(n + 1) * 128],
                        ctxT_sb[:, 2, t * 512:(t + 1) * 512],
                        start=True, stop=True,
                    )
                ot = opool.tile([128, 2, 512], BF16, tag="ot", name=f"ot{t}{k}")
                nc.vector.tensor_add(
                    out=ot[:],
                    in0=ps.rearrange("p (i c) -> p i c", i=2),
                    in1=stage_sb[:, 2 * k:2 * k + 2, t * 512:(t + 1) * 512],
                )
                nc.sync.dma_start(
                    outT_v[:, 2 * k:2 * k + 2, t * 512:(t + 1) * 512],
                    ot[:],
                )

            # ---- attention for pair p, query tile t ----
            # split_tail: for the final call, columns [0:256) of the ctx
            # accumulator are final after ik=13 (iks 14,15 only write higher
            # columns), so normalize + pair-2 out-projection + DMA for that
            # column half run inside the call's ACT-paced slack; only the
            # upper half remains as tail work.
            def attention(p, t, fillers, split_tail=False):
                q0 = SC * t
                nik = 4 * (t + 1)
                A = cpool.tile([128, 512], F32, tag="ctxA", name=f"A{p}{t}")
                Bd = cpool.tile([128, 512], F32, tag="ctxB", name=f"B{p}{t}")

                def emit_scores(ik):
                    sq0 = max(q0, 128 * ik)
                    W = q0 + SC - sq0
                    diag = 128 * ik >= q0
                    s_ps = spool.tile([128, 1024], F32, tag="work",
                                      name=f"s{p}{t}{ik}")
                    for half in range(2):
                        hp = slice(64 * half, 64 * half + 64)
                        nc.tensor.matmul(
                            s_ps[:, 512 * half:512 * half + W],
                            kT_sb[hp, p, ik * 128:(ik + 1) * 128],
                            qT_sb[hp, p, sq0:sq0 + W],
                            start=True, stop=not diag,
                            skip_group_check=True,
                        )
                    if diag:
                        # diagonal block: add -30000 to causally-masked
                        # entries (exp -> 0) via one I.T @ mneg matmul per head
                        for half in range(2):
                            nc.tensor.matmul(
                                s_ps[:, 512 * half:512 * half + 128],
                                ident_sb, mneg_sb,
                                start=False, stop=True,
                                skip_group_check=True,
                            )
                    e = epool.tile([128, 1024], BF16, tag="e", name=f"e{p}{t}{ik}")
                    nc.scalar.activation(
                        e.rearrange("p (h c) -> p h c", h=2)[:, :, 0:W],
                        s_ps.rearrange("p (h c) -> p h c", h=2)[:, :, 0:W],
                        mybir.ActivationFunctionType.Exp, scale=float(SCALE),
                    )
                    return e

                e_next = emit_scores(0)
                for ik in range(nik):
                    e = e_next
                    if ik + 1 < nik:
                        e_next = emit_scores(ik + 1)
                    if fillers:
                        npop = -(-len(fillers) // (nik - ik))
                        for _ in range(npop):
                            fillers.pop(0)()
                    sq0 = max(q0, 128 * ik)
                    W = q0 + SC - sq0
                    off = sq0 - q0
                    for half in range(2):
                        h = 2 * p + half
                        nc.tensor.matmul(
                            A[64 * half:64 * half + 64, off:off + W],
                            v_sb[:, ik, 64 * h:64 * h + 64],
                            e[:, 512 * half:512 * half + W],
                            start=(ik == 0), stop=(ik == nik - 1),
                            skip_group_check=True,
                        )
                    for half in range(2):
                        nc.tensor.matmul(
                            Bd[64 * half:64 * half + 64, off:off + W],
                            ones_sb,
                            e[:, 512 * half:512 * half + W],
                            start=(ik == 0), stop=(ik == nik - 1),
                            skip_group_check=True,
                        )

                    if ik == nik - 2:
                        # columns [0:256) are final (the last two key blocks
                        # only write higher columns): normalize them now so
                        # only a 256-wide normalize remains at the boundary
                        if split_tail:
                            norm_and_out2(p, A, Bd, q0, 0)
                        else:
                            norm_half(p, A, Bd, q0, 0)
                if split_tail:
                    norm_and_out2(p, A, Bd, q0, 1)
                else:
                    norm_half(p, A, Bd, q0, 1)

            def norm_half(p, A, Bd, q0, h):
                cs = slice(256 * h, 256 * h + 256)
                rcp = rpool.tile([128, 256], F32, tag="rcpq", name=f"rn{p}{h}")
                nc.vector.reciprocal_approx_fast(rcp[:], Bd[:, cs])
                nc.vector.tensor_mul(ctxT_sb[:, p, q0 + 256 * h:q0 + 256 * h + 256],
                                     A[:, cs], rcp[:])

            def norm_and_out2(p, A, Bd, q0, h):
                cs = slice(256 * h, 256 * h + 256)
                g0 = q0 + 256 * h
                rcp = rpool.tile([128, 256], F32, tag="rcpq", name=f"rq{h}")
                nc.vector.reciprocal_approx_fast(rcp[:], Bd[:, cs])
                nc.vector.tensor_mul(ctxT_sb[:, p, g0:g0 + 256], A[:, cs], rcp[:])
                for k in range(3):
                    ps = spool.tile([128, 1024], F32, tag="work", name=f"oq{h}{k}")
                    for i in range(2):
                        n = 2 * k + i
                        nc.tensor.matmul(
                            ps[:, i * 512:i * 512 + 256],
                            wo_sb[:, 2, n * 128:(n + 1) * 128],
                            ctxT_sb[:, 2, g0:g0 + 256],
                            start=True, stop=True,
                        )
                    ot = opool.tile([128, 2, 256], BF16, tag="otq", name=f"otq{h}{k}")
                    nc.vector.tensor_add(
                        out=ot[:],
                        in0=ps.rearrange("p (i c) -> p i c", i=2)[:, :, 0:256],
                        in1=stage_sb[:, 2 * k:2 * k + 2, g0:g0 + 256],
                    )
                    eng = nc.sync if k % 2 == 0 else nc.scalar
                    eng.dma_start(outT_v[:, 2 * k:2 * k + 2, g0:g0 + 256], ot[:])

            # ---- emission schedule ----
            import functools as ft
            P = ft.partial

            # prologue: minimum needed by attention(0,0) / early (0,1)
            qk_sub(0, 0, 0)
            qk_sub(0, 1, 0)
            v_unit(0)
            v_unit(1)

            # early windows: fillers ordered by input-DMA arrival so the
            # attention pipeline-fill gaps are covered by ready work
            # (pair-1 t=0 projections need only the first xT piece + wq/wk)
            FILL = {
                (0, 0): [P(qk_sub, 1, 0, 0), P(qk_sub, 0, 0, 1),
                         P(qk_sub, 0, 1, 1), P(qk_sub, 1, 1, 0)],
                (0, 1): [P(v_unit, 2), P(v_unit, 3), P(qk_unit, 0, 0, 1)],
                (0, 2): [P(qk_unit, 0, 1, 1), P(qk_sub, 1, 0, 1),
                         P(v_unit, 4), P(v_unit, 5)],
                (0, 3): [P(v_unit, 6), P(v_unit, 7), P(qk_sub, 1, 1, 1),
                         P(qk_unit, 1, 0, 1)],
                (1, 0): [P(qk_unit, 1, 1, 1), P(qk_unit, 2, 0, 0)],
                (1, 1): [P(qk_unit, 2, 1, 0), P(qk_unit, 2, 0, 1)],
                (1, 2): [P(qk_unit, 2, 1, 1), P(o01_unit, 0, 0),
                         P(o01_unit, 0, 1), P(o01_unit, 0, 2)],
                (1, 3): [P(o01_unit, 1, 0), P(o01_unit, 1, 1),
                         P(o01_unit, 1, 2), P(o01_unit, 2, 0)],
                (2, 0): [P(o01_unit, 2, 1), P(o01_unit, 2, 2)],
                (2, 1): [P(o2_unit, 0, 0), P(o2_unit, 0, 1), P(o2_unit, 0, 2),
                         P(o01_unit, 3, 0), P(o01_unit, 3, 1)],
                (2, 2): [P(o01_unit, 3, 2), P(o2_unit, 1, 0),
                         P(o2_unit, 1, 1), P(o2_unit, 1, 2)],
                (2, 3): [P(o2_unit, 2, 0), P(o2_unit, 2, 1), P(o2_unit, 2, 2)],
            }
            for p in range(NPAIR):
                for t in range(NT):
                    attention(p, t, FILL[(p, t)],
                              split_tail=(p == 2 and t == 3))
    nc.finalize()
    return nc


_NC_CACHE = None


def _get_nc():
    global _NC_CACHE
    if _NC_CACHE is None:
        _NC_CACHE = build_nc()
    return _NC_CACHE


def make_in_maps(x, Wq, Wk, Wv, bq, bk, bv, Wo, bo):
    bf16 = ml_dtypes.bfloat16
    ident = np.eye(128, dtype=np.float32)
    # mneg[sk, sq] = -30000 where sq < sk (causally masked in the diagonal)
    mneg = np.where(np.arange(128)[None, :] < np.arange(128)[:, None],
                    np.float32(-30000.0), np.float32(0.0))
    ones64 = np.ones((128, 64), np.float32)
    const = np.ascontiguousarray(
        np.concatenate([ident, mneg, ones64], axis=1)).astype(bf16)

    def prep_w(W, hs):
        # [H, 768, 64] -> [768, H*64] -> [128, NC_D, 384] -> [128, NC_D*384]
        w = np.asarray(W[hs]).transpose(1, 0, 2).reshape(D, H * HD)
        w = w.reshape(NC_D, 128, H * HD).transpose(1, 0, 2).reshape(128, -1)
        return np.ascontiguousarray(w).astype(bf16)

    in_maps = []
    for core in range(8):
        b, g = core // 2, core % 2
        hs = slice(6 * g, 6 * g + 6)
        xT = np.ascontiguousarray(np.asarray(x[b]).T).astype(bf16)
        bqk = np.zeros((128, 2 * NPAIR), np.float32)
        for p in range(NPAIR):
            bqk[0:64, p] = bq[6 * g + 2 * p]
            bqk[64:128, p] = bq[6 * g + 2 * p + 1]
            bqk[0:64, NPAIR + p] = bk[6 * g + 2 * p]
            bqk[64:128, NPAIR + p] = bk[6 * g + 2 * p + 1]
        bvr = np.ascontiguousarray(
            np.broadcast_to(np.asarray(bv[hs]).reshape(1, H * HD), (128, H * HD))
        ).astype(np.float32)
        wo = np.asarray(Wo[384 * g:384 * (g + 1), :])
        wo = wo.reshape(3, 128, D).transpose(1, 0, 2).reshape(128, -1)
        wo = np.ascontiguousarray(wo).astype(bf16)
        in_maps.append({
            "xT": xT,
            "wq": prep_w(Wq, hs), "wk": prep_w(Wk, hs), "wv": prep_w(Wv, hs),
            "wo": wo, "bqk": bqk, "bv": bvr, "const": const,
        })
    return in_maps


def gather_out(results, bo):
    out = np.empty((B, S, D), np.float32)
    bo32 = np.asarray(bo, np.float32)
    for b in range(B):
        pT = (results[2 * b]["outT"].astype(np.float32)
              + results[2 * b + 1]["outT"].astype(np.float32))
        out[b] = pT.T + bo32[None, :]
    return out


def kernel(x, Wq, Wk, Wv, bq, bk, bv, Wo, bo):
    from concourse.bass_utils import run_bass_kernel_spmd

    nc = _get_nc()
    in_maps = make_in_maps(x, Wq, Wk, Wv, bq, bk, bv, Wo, bo)
    res = run_bass_kernel_spmd(nc, in_maps, list(range(8)))
    return gather_out(res.results, bo)


# revision 31
# speedup vs baseline: 1.0388x; 1.0193x over previous
"""
Multi-head masked (causal) attention on 8 Trainium2 NeuronCores.

Sharding: core = 2*b + g  (b = batch 0..3, g = head-group 0..1, 6 heads each).
Each core computes, for its batch b and heads [6g, 6g+6):
    q,k,v projections -> causal attention -> out-projection rows
    [384g, 384g+384) of Wo, output written TRANSPOSED [768, 2048] bf16.
Host gathers: out[b] = (part[2b] + part[2b+1]).T + bo.

Attention is processed per head-PAIR p (3 pairs) and per 512-wide query
tile t (4 tiles).  Scores are computed transposed (S^T[sk, sq]) with the
two heads of a pair occupying PE row-groups 0-1 / 2-3 concurrently
(K=64 each), written into ONE [128, 1024] PSUM tile (half h at columns
[512h, 512h+W)), so a single ACT exp instruction covers both heads.
AV and the softmax-denominator matmuls are M=64:
    A[0:64]  = ctx head0   A[64:128] = ctx head1
    B[0:64]  = denom head0 B[64:128] = denom head1
so normalization is one full-width reciprocal + one full-width multiply
straight out of PSUM (no copies, no partition-shifted PSUM reads).

PSUM budget (8 banks): score pipeline 3 x [128,1024] f32 (6 banks,
shared with projection/out-proj filler units) + A + B (2 banks).

Out-projection: contributions of pairs 0,1 are staged in SBUF (bf16)
as PE filler work; pair-2 contribution is added on top (DVE) and the
single bf16 result is DMA'd out per (t, n-pair) tile.
"""

import numpy as np
import ml_dtypes

import concourse.bass as bass
import concourse.mybir as mybir
import concourse.tile as tile
from concourse import bacc

BF16 = mybir.dt.bfloat16
F32 = mybir.dt.float32

# Problem constants (hardcoded per contract)
B, S, D = 4, 2048, 768
N_HEADS_TOTAL = 12
HD = 64                      # head dim
H = 6                        # local heads per core
NPAIR = H // 2               # head pairs
NC_D = D // 128              # contraction chunks over D (6)
NSK = S // 128               # key blocks (16)
SC = 512                     # query-tile width
NT = S // SC                 # query tiles (4)
SCALE = 1.0 / np.sqrt(HD)


def build_nc():
    nc = bacc.Bacc(None, target_bir_lowering=False)

    xT_d = nc.declare_dram_parameter("xT", [D, S], BF16, isOutput=False)
    wq_d = nc.declare_dram_parameter("wq", [128, NC_D * 384], BF16, isOutput=False)
    wk_d = nc.declare_dram_parameter("wk", [128, NC_D * 384], BF16, isOutput=False)
    wv_d = nc.declare_dram_parameter("wv", [128, NC_D * 384], BF16, isOutput=False)
    wo_d = nc.declare_dram_parameter("wo", [128, 3 * 768], BF16, isOutput=False)
    bqk_d = nc.declare_dram_parameter("bqk", [128, 2 * NPAIR], F32, isOutput=False)
    bv_d = nc.declare_dram_parameter("bv", [128, 384], F32, isOutput=False)
    # [128, 320] = identity(128) | mneg(128) | ones(64)
    const_d = nc.declare_dram_parameter("const", [128, 320], BF16, isOutput=False)
    outT_d = nc.declare_dram_parameter("outT", [D, S], BF16, isOutput=True)
    outT_v = outT_d.rearrange("(k p) c -> p k c", p=128)

    with tile.TileContext(nc) as tc:
        with (
            tc.tile_pool(name="const", bufs=1) as constp,
            tc.tile_pool(name="big", bufs=1) as bigp,
            tc.tile_pool(name="epool", bufs=5) as epool,
            tc.tile_pool(name="rpool", bufs=2) as rpool,
            tc.tile_pool(name="opool", bufs=3) as opool,
            tc.tile_pool(name="spool", bufs=3, space="PSUM") as spool,
            tc.tile_pool(name="cpool", bufs=1, space="PSUM") as cpool,
        ):
            xT_sb = bigp.tile([128, NC_D, S], BF16)
            qT_sb = bigp.tile([128, NPAIR, S], BF16)
            kT_sb = bigp.tile([128, NPAIR, S], BF16)
            v_sb = bigp.tile([128, NSK, H * HD], BF16)
            ctxT_sb = bigp.tile([128, NPAIR, S], BF16)
            stage_sb = bigp.tile([128, D // 128, S], BF16)
            wq_sb = constp.tile([128, NC_D, 384], BF16)
            wk_sb = constp.tile([128, NC_D, 384], BF16)
            wv_sb = constp.tile([128, NC_D, 384], BF16)
            wo_sb = constp.tile([128, 3, 768], BF16)
            bqk_sb = constp.tile([128, 2 * NPAIR], F32)
            bv_sb = constp.tile([128, 384], F32)
            const_sb = constp.tile([128, 320], BF16)
            ident_sb = const_sb[:, 0:128]
            mneg_sb = const_sb[:, 128:256]
            ones_sb = const_sb[:, 256:320]

            # ---- input DMAs.  sync: xT (first 512 cols fine-grained, rest
            # bulk).  scalar: weights/consts, most-urgent first.
            for c in range(NC_D):
                nc.sync.dma_start(xT_sb[:, c, 0:512],
                                  xT_d[c * 128:(c + 1) * 128, 0:512])
            # tiny tensors first (their transfer time is negligible but
            # queued between the big weights they delay wk by several us:
            # the ring drains descriptors strictly in order while xT steals
            # ~half the HBM bandwidth), then weights in consumption order
            nc.scalar.dma_start(bqk_sb[:], bqk_d[:])
            nc.scalar.dma_start(const_sb[:], const_d[:])
            nc.scalar.dma_start(wq_sb[:], wq_d.rearrange("p (c n) -> p c n", n=384))
            nc.scalar.dma_start(wk_sb[:], wk_d.rearrange("p (c n) -> p c n", n=384))
            nc.scalar.dma_start(wv_sb[:], wv_d.rearrange("p (c n) -> p c n", n=384))
            for c in range(NC_D):
                nc.sync.dma_start(xT_sb[:, c, 512:1024],
                                  xT_d[c * 128:(c + 1) * 128, 512:1024])
            nc.scalar.dma_start(bv_sb[:], bv_d[:])
            nc.scalar.dma_start(wo_sb[:], wo_d.rearrange("p (c n) -> p c n", n=768))
            for c in range(NC_D):
                nc.sync.dma_start(xT_sb[:, c, 1024:S],
                                  xT_d[c * 128:(c + 1) * 128, 1024:S])

            # ---- projection / out-projection units (PE filler work) ----
            def qk_sub(p, which, t):
                """q or k projection for pair p, 512 query cols."""
                w_sb, dst = ((wq_sb, qT_sb), (wk_sb, kT_sb))[which]
                ps = spool.tile([128, 1024], F32, tag="work", name=f"qks{p}{which}{t}")
                for c in range(NC_D):
                    nc.tensor.matmul(
                        ps[:, 0:512],
                        w_sb[:, c, p * 128:(p + 1) * 128],
                        xT_sb[:, c, t * 512:(t + 1) * 512],
                        start=(c == 0), stop=(c == NC_D - 1),
                    )
                nc.vector.tensor_add(
                    out=dst[:, p, t * 512:(t + 1) * 512],
                    in0=ps[:, 0:512],
                    in1=bqk_sb[:, 3 * which + p:3 * which + p + 1]
                        .broadcast_to((128, 512)),
                )

            def qk_unit(p, which, tp):
                """q or k projection for pair p, 1024 query cols (2 chunks)."""
                w_sb, dst = ((wq_sb, qT_sb), (wk_sb, kT_sb))[which]
                ps = spool.tile([128, 1024], F32, tag="work", name=f"qk{p}{which}{tp}")
                base = 1024 * tp
                for i in range(2):
                    for c in range(NC_D):
                        nc.tensor.matmul(
                            ps[:, i * 512:(i + 1) * 512],
                            w_sb[:, c, p * 128:(p + 1) * 128],
                            xT_sb[:, c, base + i * 512:base + i * 512 + 512],
                            start=(c == 0), stop=(c == NC_D - 1),
                        )
                nc.vector.tensor_add(
                    out=dst[:, p, base:base + 1024],
                    in0=ps[:, 0:1024],
                    in1=bqk_sb[:, 3 * which + p:3 * which + p + 1]
                        .broadcast_to((128, 1024)),
                )

            def v_unit(j):
                """v projection for token blocks 2j, 2j+1."""
                ps = spool.tile([128, 1024], F32, tag="work", name=f"v{j}")
                for i in range(2):
                    s = 2 * j + i
                    for c in range(NC_D):
                        nc.tensor.matmul(
                            ps[:, i * 512:i * 512 + 384],
                            xT_sb[:, c, s * 128:(s + 1) * 128],
                            wv_sb[:, c, :],
                            start=(c == 0), stop=(c == NC_D - 1),
                        )
                for i in range(2):
                    nc.vector.tensor_add(
                        out=v_sb[:, 2 * j + i, :],
                        in0=ps[:, i * 512:i * 512 + 384],
                        in1=bv_sb[:],
                    )

            def o01_unit(t, k):
                """stage out-proj contribution of pairs 0,1 for n = 2k, 2k+1."""
                ps = spool.tile([128, 1024], F32, tag="work", name=f"o01_{t}{k}")
                for i in range(2):
                    n = 2 * k + i
                    for c in range(2):
                        nc.tensor.matmul(
                            ps[:, i * 512:(i + 1) * 512],
                            wo_sb[:, c, n * 128:(n + 1) * 128],
                            ctxT_sb[:, c, t * 512:(t + 1) * 512],
                            start=(c == 0), stop=(c == 1),
                        )
                nc.vector.tensor_copy(
                    stage_sb[:, 2 * k:2 * k + 2, t * 512:(t + 1) * 512],
                    ps.rearrange("p (i c) -> p i c", i=2),
                )

            def o2_unit(t, k):
                """add pair-2 contribution onto the stage and DMA out."""
                ps = spool.tile([128, 1024], F32, tag="work", name=f"o2_{t}{k}")
                for i in range(2):
                    n = 2 * k + i
                    nc.tensor.matmul(
                        ps[:, i * 512:(i + 1) * 512],
                        wo_sb[:, 2, n * 128:(n + 1) * 128],
                        ctxT_sb[:, 2, t * 512:(t + 1) * 512],
                        start=True, stop=True,
                    )
                ot = opool.tile([128, 2, 512], BF16, tag="ot", name=f"ot{t}{k}")
                nc.vector.tensor_add(
                    out=ot[:],
                    in0=ps.rearrange("p (i c) -> p i c", i=2),
                    in1=stage_sb[:, 2 * k:2 * k + 2, t * 512:(t + 1) * 512],
                )
                nc.sync.dma_start(
                    outT_v[:, 2 * k:2 * k + 2, t * 512:(t + 1) * 512],
                    ot[:],
                )

            # ---- attention for pair p, query tile t ----
            # split_tail: for the final call, columns [0:256) of the ctx
            # accumulator are final after ik=13 (iks 14,15 only write higher
            # columns), so normalize + pair-2 out-projection + DMA for that
            # column half run inside the call's ACT-paced slack; only the
            # upper half remains as tail work.
            def attention(p, t, fillers, split_tail=False):
                q0 = SC * t
                nik = 4 * (t + 1)
                A = cpool.tile([128, 512], F32, tag="ctxA", name=f"A{p}{t}")
                Bd = cpool.tile([128, 512], F32, tag="ctxB", name=f"B{p}{t}")

                def emit_scores(ik):
                    sq0 = max(q0, 128 * ik)
                    W = q0 + SC - sq0
                    diag = 128 * ik >= q0
                    s_ps = spool.tile([128, 1024], F32, tag="work",
                                      name=f"s{p}{t}{ik}")
                    for half in range(2):
                        hp = slice(64 * half, 64 * half + 64)
                        nc.tensor.matmul(
                            s_ps[:, 512 * half:512 * half + W],
                            kT_sb[hp, p, ik * 128:(ik + 1) * 128],
                            qT_sb[hp, p, sq0:sq0 + W],
                            start=True, stop=not diag,
                            skip_group_check=True,
                        )
                    if diag:
                        # diagonal block: add -30000 to causally-masked
                        # entries (exp -> 0) via one I.T @ mneg matmul per head
                        for half in range(2):
                            nc.tensor.matmul(
                                s_ps[:, 512 * half:512 * half + 128],
                                ident_sb, mneg_sb,
                                start=False, stop=True,
                                skip_group_check=True,
                            )
                    e = epool.tile([128, 1024], BF16, tag="e", name=f"e{p}{t}{ik}")
                    nc.scalar.activation(
                        e.rearrange("p (h c) -> p h c", h=2)[:, :, 0:W],
                        s_ps.rearrange("p (h c) -> p h c", h=2)[:, :, 0:W],
                        mybir.ActivationFunctionType.Exp, scale=float(SCALE),
                    )
                    return e

                e_next = emit_scores(0)
                for ik in range(nik):
                    e = e_next
                    if ik + 1 < nik:
                        e_next = emit_scores(ik + 1)
                    if fillers:
                        npop = -(-len(fillers) // (nik - ik))
                        for _ in range(npop):
                            fillers.pop(0)()
                    sq0 = max(q0, 128 * ik)
                    W = q0 + SC - sq0
                    off = sq0 - q0
                    for half in range(2):
                        h = 2 * p + half
                        nc.tensor.matmul(
                            A[64 * half:64 * half + 64, off:off + W],
                            v_sb[:, ik, 64 * h:64 * h + 64],
                            e[:, 512 * half:512 * half + W],
                            start=(ik == 0), stop=(ik == nik - 1),
                            skip_group_check=True,
                        )
                    for half in range(2):
                        nc.tensor.matmul(
                            Bd[64 * half:64 * half + 64, off:off + W],
                            ones_sb,
                            e[:, 512 * half:512 * half + W],
                            start=(ik == 0), stop=(ik == nik - 1),
                            skip_group_check=True,
                        )

                    if ik == nik - 2:
                        # columns [0:256) are final (the last two key blocks
                        # only write higher columns): normalize them now so
                        # only a 256-wide normalize remains at the boundary
                        if split_tail:
                            norm_and_out2(p, A, Bd, q0, 0)
                        else:
                            norm_half(p, A, Bd, q0, 0)
                if split_tail:
                    norm_and_out2(p, A, Bd, q0, 1)
                else:
                    norm_half(p, A, Bd, q0, 1)

            def norm_half(p, A, Bd, q0, h):
                cs = slice(256 * h, 256 * h + 256)
                rcp = rpool.tile([128, 256], F32, tag="rcpq", name=f"rn{p}{h}")
                nc.vector.reciprocal_approx_fast(rcp[:], Bd[:, cs])
                nc.vector.tensor_mul(ctxT_sb[:, p, q0 + 256 * h:q0 + 256 * h + 256],
                                     A[:, cs], rcp[:])

            def norm_and_out2(p, A, Bd, q0, h):
                cs = slice(256 * h, 256 * h + 256)
                g0 = q0 + 256 * h
                rcp = rpool.tile([128, 256], F32, tag="rcpq", name=f"rq{h}")
                nc.vector.reciprocal_approx_fast(rcp[:], Bd[:, cs])
                nc.vector.tensor_mul(ctxT_sb[:, p, g0:g0 + 256], A[:, cs], rcp[:])
                for k in range(3):
                    ps = spool.tile([128, 1024], F32, tag="work", name=f"oq{h}{k}")
                    for i in range(2):
                        n = 2 * k + i
                        nc.tensor.matmul(
                            ps[:, i * 512:i * 512 + 256],
                            wo_sb[:, 2, n * 128:(n + 1) * 128],
                            ctxT_sb[:, 2, g0:g0 + 256],
                            start=True, stop=True,
                        )
                    ot = opool.tile([128, 2, 256], BF16, tag="otq", name=f"otq{h}{k}")
                    nc.vector.tensor_add(
                        out=ot[:],
                        in0=ps.rearrange("p (i c) -> p i c", i=2)[:, :, 0:256],
                        in1=stage_sb[:, 2 * k:2 * k + 2, g0:g0 + 256],
                    )
                    eng = nc.sync if k % 2 == 0 else nc.scalar
                    eng.dma_start(outT_v[:, 2 * k:2 * k + 2, g0:g0 + 256], ot[:])

            # ---- emission schedule ----
            import functools as ft
            P = ft.partial

            # prologue: minimum needed by attention(0,0) / early (0,1)
            qk_sub(0, 0, 0)
            qk_sub(0, 1, 0)
            v_unit(0)
            v_unit(1)

            # early windows: fillers ordered by input-DMA arrival so the
            # attention pipeline-fill gaps are covered by ready work
            # (pair-1 t=0 projections need only the first xT piece + wq/wk)
            FILL = {
                (0, 0): [P(qk_sub, 1, 0, 0), P(qk_sub, 0, 0, 1),
                         P(qk_sub, 0, 1, 1), P(qk_sub, 1, 1, 0)],
                (0, 1): [P(v_unit, 2), P(v_unit, 3), P(qk_unit, 0, 0, 1)],
                (0, 2): [P(qk_unit, 0, 1, 1), P(qk_sub, 1, 0, 1),
                         P(v_unit, 4), P(v_unit, 5)],
                (0, 3): [P(v_unit, 6), P(v_unit, 7), P(qk_sub, 1, 1, 1),
                         P(qk_unit, 1, 0, 1)],
                (1, 0): [P(qk_unit, 1, 1, 1), P(qk_unit, 2, 0, 0)],
                (1, 1): [P(qk_unit, 2, 1, 0), P(qk_unit, 2, 0, 1)],
                (1, 2): [P(qk_unit, 2, 1, 1), P(o01_unit, 0, 0),
                         P(o01_unit, 0, 1), P(o01_unit, 0, 2)],
                (1, 3): [P(o01_unit, 1, 0), P(o01_unit, 1, 1),
                         P(o01_unit, 1, 2), P(o01_unit, 2, 0)],
                (2, 0): [P(o01_unit, 2, 1), P(o01_unit, 2, 2)],
                (2, 1): [P(o2_unit, 0, 0), P(o2_unit, 0, 1), P(o2_unit, 0, 2),
                         P(o01_unit, 3, 0), P(o01_unit, 3, 1)],
                (2, 2): [P(o01_unit, 3, 2), P(o2_unit, 1, 0),
                         P(o2_unit, 1, 1), P(o2_unit, 1, 2)],
                (2, 3): [P(o2_unit, 2, 0), P(o2_unit, 2, 1), P(o2_unit, 2, 2)],
            }
            for p in range(NPAIR):
                for t in range(NT):
                    attention(p, t, FILL[(p, t)],
                              split_tail=(p == 2 and t == 3))
    nc.finalize()
    return nc


_NC_CACHE = None


def _get_nc():
    global _NC_CACHE
    if _NC_CACHE is None:
        _NC_CACHE = build_nc()
    return _NC_CACHE


def make_in_maps(x, Wq, Wk, Wv, bq, bk, bv, Wo, bo):
    bf16 = ml_dtypes.bfloat16
    ident = np.eye(128, dtype=np.float32)
    # mneg[sk, sq] = -30000 where sq < sk (causally masked in the diagonal)
    mneg = np.where(np.arange(128)[None, :] < np.arange(128)[:, None],
                    np.float32(-30000.0), np.float32(0.0))
    ones64 = np.ones((128, 64), np.float32)
    const = np.ascontiguousarray(
        np.concatenate([ident, mneg, ones64], axis=1)).astype(bf16)

    def prep_w(W, hs):
        # [H, 768, 64] -> [768, H*64] -> [128, NC_D, 384] -> [128, NC_D*384]
        w = np.asarray(W[hs]).transpose(1, 0, 2).reshape(D, H * HD)
        w = w.reshape(NC_D, 128, H * HD).transpose(1, 0, 2).reshape(128, -1)
        return np.ascontiguousarray(w).astype(bf16)

    in_maps = []
    for core in range(8):
        b, g = core // 2, core % 2
        hs = slice(6 * g, 6 * g + 6)
        xT = np.ascontiguousarray(np.asarray(x[b]).T).astype(bf16)
        bqk = np.zeros((128, 2 * NPAIR), np.float32)
        for p in range(NPAIR):
            bqk[0:64, p] = bq[6 * g + 2 * p]
            bqk[64:128, p] = bq[6 * g + 2 * p + 1]
            bqk[0:64, NPAIR + p] = bk[6 * g + 2 * p]
            bqk[64:128, NPAIR + p] = bk[6 * g + 2 * p + 1]
        bvr = np.ascontiguousarray(
            np.broadcast_to(np.asarray(bv[hs]).reshape(1, H * HD), (128, H * HD))
        ).astype(np.float32)
        wo = np.asarray(Wo[384 * g:384 * (g + 1), :])
        wo = wo.reshape(3, 128, D).transpose(1, 0, 2).reshape(128, -1)
        wo = np.ascontiguousarray(wo).astype(bf16)
        in_maps.append({
            "xT": xT,
            "wq": prep_w(Wq, hs), "wk": prep_w(Wk, hs), "wv": prep_w(Wv, hs),
            "wo": wo, "bqk": bqk, "bv": bvr, "const": const,
        })
    return in_maps


def gather_out(results, bo):
    out = np.empty((B, S, D), np.float32)
    bo32 = np.asarray(bo, np.float32)
    for b in range(B):
        pT = (results[2 * b]["outT"].astype(np.float32)
              + results[2 * b + 1]["outT"].astype(np.float32))
        out[b] = pT.T + bo32[None, :]
    return out


def kernel(x, Wq, Wk, Wv, bq, bk, bv, Wo, bo):
    from concourse.bass_utils import run_bass_kernel_spmd

    nc = _get_nc()
    in_maps = make_in_maps(x, Wq, Wk, Wv, bq, bk, bv, Wo, bo)
    res = run_bass_kernel_spmd(nc, in_maps, list(range(8)))
    return gather_out(res.results, bo)
